# revision 8
# baseline (speedup 1.0000x reference)
"""AttentionBlock (GroupNorm32 + self/cross attention + proj + residual) on 8 TRN2 cores.

Sharding: data-parallel over batch. B=8 samples, one per NeuronCore. Each core runs
the full per-sample block: GroupNorm -> qkv/ekv projections -> 8-head attention
(encoder kv concat + additive mask) -> out projection -> residual.

Layout notes (per core):
  - channel-major tensors are stored as [128, 4, *]  with c = 128*i + p
  - attention logits are computed TRANSPOSED, wgtT[s, t], so the (per-s) additive
    mask becomes a per-partition ACT bias and softmax's exp is a single
    PSUM->SBUF activation pass; row sums come from an appended ones-column in
    v^T during the PV matmul (out row 64).
  - matmuls run in bf16 (fp32 PSUM accumulate).
"""

import sys
from contextlib import ExitStack

import numpy as np

for _p in ("/opt/trn_rl_repo",):
    if _p not in sys.path:
        sys.path.insert(0, _p)

import ml_dtypes  # noqa: E402

import concourse.bass as bass  # noqa: E402
import concourse.tile as tile  # noqa: E402
from concourse import bacc, mybir  # noqa: E402
from concourse.masks import make_identity  # noqa: E402

F32 = mybir.dt.float32
BF16 = mybir.dt.bfloat16
AF = mybir.ActivationFunctionType
ALU = mybir.AluOpType

B, C, HH, WW = 8, 512, 32, 32
T = HH * WW          # 1024
HEADS, CH, S = 8, 64, 77
ST = S + T           # 1101
NS = (ST + 127) // 128  # 9 s-tiles
GROUPS, GCH = 32, 16
N_CORES = 8

# v^T per head is [s, 65]: cols 0..63 = v channels, col 64 = ones (rowsum trick).
# Two heads per pair live side by side in one tile with a 66-col stride.
VTC = 66


def build_program():
    nc = bacc.Bacc("TRN2", target_bir_lowering=False, debug=False)

    x_d = nc.dram_tensor("x", [128, 4, T], F32, kind="ExternalInput")
    enc_d = nc.dram_tensor("enc", [128, 4, S], BF16, kind="ExternalInput")
    addm_d = nc.dram_tensor("addm", [128, 1], F32, kind="ExternalInput")
    wq_d = nc.dram_tensor("wq", [128, 4, 3 * C], BF16, kind="ExternalInput")
    we_d = nc.dram_tensor("we", [128, 4, 2 * C], BF16, kind="ExternalInput")
    wp_d = nc.dram_tensor("wp", [128, 4, C], BF16, kind="ExternalInput")
    qb_d = nc.dram_tensor("qb", [128, 4, 3], F32, kind="ExternalInput")
    eb_d = nc.dram_tensor("eb", [128, 4, 2], F32, kind="ExternalInput")
    pb_d = nc.dram_tensor("pb", [128, 4], F32, kind="ExternalInput")
    gam_d = nc.dram_tensor("gam", [128, 4], F32, kind="ExternalInput")
    bet_d = nc.dram_tensor("bet", [128, 4], F32, kind="ExternalInput")
    out_d = nc.dram_tensor("out", [128, 4, T], F32, kind="ExternalOutput")

    with tile.TileContext(nc) as tc, ExitStack() as ctx:
        consts = ctx.enter_context(tc.tile_pool(name="consts", bufs=1))
        stats = ctx.enter_context(tc.tile_pool(name="stats", bufs=1))
        hp = ctx.enter_context(tc.tile_pool(name="hp", bufs=2))
        psum = ctx.enter_context(tc.tile_pool(name="psum", bufs=1, space="PSUM"))

        # ---- constant loads -------------------------------------------------
        x_sb = consts.tile([128, 4, T], F32)
        nc.sync.dma_start(out=x_sb, in_=x_d.ap())
        enc_sb = consts.tile([128, 4, S], BF16)
        nc.sync.dma_start(out=enc_sb, in_=enc_d.ap())
        wq = consts.tile([128, 4, 3 * C], BF16)
        nc.sync.dma_start(out=wq, in_=wq_d.ap())
        we = consts.tile([128, 4, 2 * C], BF16)
        nc.sync.dma_start(out=we, in_=we_d.ap())
        wp = consts.tile([128, 4, C], BF16)
        nc.sync.dma_start(out=wp, in_=wp_d.ap())
        addm = consts.tile([128, 1], F32)
        nc.sync.dma_start(out=addm, in_=addm_d.ap())
        qb = consts.tile([128, 4, 3], F32)
        nc.sync.dma_start(out=qb, in_=qb_d.ap())
        eb = consts.tile([128, 4, 2], F32)
        nc.sync.dma_start(out=eb, in_=eb_d.ap())
        pb = consts.tile([128, 4], F32)
        nc.sync.dma_start(out=pb, in_=pb_d.ap())
        gam = consts.tile([128, 4], F32)
        nc.sync.dma_start(out=gam, in_=gam_d.ap())
        bet = consts.tile([128, 4], F32)
        nc.sync.dma_start(out=bet, in_=bet_d.ap())

        ident = consts.tile([128, 128], BF16)
        make_identity(nc, ident)
        identf = consts.tile([128, 128], F32)
        make_identity(nc, identf)

        # ---- GroupNorm(32) stats -------------------------------------------
        # per-channel mean/var via bn_stats, then aggregate channel groups of 16
        # across partitions via a PE transpose.
        mv = stats.tile([128, 4, 2], F32)
        for i in range(4):
            bnst = stats.tile([128, 2, 6], F32, tag="bnst", bufs=2)
            nc.vector.bn_stats(out=bnst[:, 0, :], in_=x_sb[:, i, 0:512])
            nc.vector.bn_stats(out=bnst[:, 1, :], in_=x_sb[:, i, 512:1024])
            nc.vector.bn_aggr(out=mv[:, i, :], in_=bnst)

        stm = stats.tile([128, 4], F32)
        nc.vector.tensor_copy(out=stm, in_=mv[:, :, 0])
        # E[x^2] = var + mean^2
        stx = stats.tile([128, 4], F32)
        nc.vector.tensor_mul(out=stx, in0=mv[:, :, 0], in1=mv[:, :, 0])
        nc.vector.tensor_add(out=stx, in0=stx, in1=mv[:, :, 1])

        stmT = stats.tile([4, 128], F32)
        stxT = stats.tile([4, 128], F32)
        for src_t, dst_t in ((stm, stmT), (stx, stxT)):
            t_ps = psum.tile([4, 128], F32, tag="mm", bufs=2, name=f"tps_{src_t.tensor.name}")
            nc.tensor.transpose(t_ps, src_t, identf)
            nc.vector.tensor_copy(out=dst_t, in_=t_ps)

        # group sums over 16 channels: [4, 8groups, 16ch] -> [4, 8]
        gm = stats.tile([4, 8], F32)
        gx = stats.tile([4, 8], F32)
        nc.vector.reduce_sum(
            out=gm, in_=stmT.rearrange("p (g k) -> p g k", k=GCH),
            axis=mybir.AxisListType.X)
        nc.vector.reduce_sum(
            out=gx, in_=stxT.rearrange("p (g k) -> p g k", k=GCH),
            axis=mybir.AxisListType.X)
        mug = stats.tile([4, 8], F32)
        nc.scalar.mul(out=mug, in_=gm, mul=1.0 / GCH)
        varg = stats.tile([4, 8], F32)
        nc.vector.tensor_mul(out=varg, in0=mug, in1=mug)
        nc.scalar.mul(out=gx, in_=gx, mul=1.0 / GCH)
        nc.vector.tensor_sub(out=varg, in0=gx, in1=varg)
        # rstd = (var+eps)^-1/2 = exp(-0.5*ln(var+eps)); keeps ACT on one table set
        eps_t = stats.tile([4, 1], F32)
        nc.vector.memset(eps_t, 1e-5)
        lnv = stats.tile([4, 8], F32)
        nc.scalar.activation(out=lnv, in_=varg, func=AF.Ln, bias=eps_t, scale=1.0)
        rstdg = stats.tile([4, 8], F32)
        nc.scalar.activation(out=rstdg, in_=lnv, func=AF.Exp, scale=-0.5)

        # transpose [4ctile, 8grp] -> [8grp, 4ctile], then replicate x16 along
        # partitions back to per-channel [128, 4] columns via a tiny DMA.
        mu_col = stats.tile([128, 4], F32)
        rstd_col = stats.tile([128, 4], F32)
        for src_t, dst_t in ((mug, mu_col), (rstdg, rstd_col)):
            t_ps = psum.tile([8, 4], F32, tag="mm", bufs=2, name=f"tbps_{src_t.tensor.name}")
            nc.tensor.transpose(t_ps, src_t, identf[0:4, 0:4])
            t_sb = stats.tile([8, 4], F32, name=f"tsb_{src_t.tensor.name}")
            nc.vector.tensor_copy(out=t_sb, in_=t_ps)
            nc.sync.dma_start(
                out=dst_t,
                in_=bass.AP(
                    tensor=t_sb.tensor, offset=t_sb.offset,
                    ap=[list(t_sb.ap[0]), [0, GCH], list(t_sb.ap[-1])],
                ),
            )

        a_col = stats.tile([128, 4], F32)
        nc.vector.tensor_mul(out=a_col, in0=rstd_col, in1=gam)
        b_col = stats.tile([128, 4], F32)
        nc.vector.tensor_mul(out=b_col, in0=mu_col, in1=a_col)
        nc.vector.tensor_sub(out=b_col, in0=bet, in1=b_col)

        nrm = consts.tile([128, 4, T], BF16)
        for i in range(4):
            nc.vector.tensor_scalar(
                out=nrm[:, i, :], in0=x_sb[:, i, :],
                scalar1=a_col[:, i:i + 1], scalar2=b_col[:, i:i + 1],
                op0=ALU.mult, op1=ALU.add,
            )

        att_all = consts.tile([128, 4, T], BF16)

        # ---- attention, two heads (one pair) at a time ----------------------
        for p in range(4):
            h0 = 2 * p
            qq2 = hp.tile([128, T], BF16, tag="qq2")
            kk2 = hp.tile([128, ST], BF16, tag="kk2")
            vv2 = hp.tile([128, ST], BF16, tag="vv2")
            vT2 = hp.tile([128, NS, 2 * VTC], BF16, tag="vT2")

            # qkv projection for the pair; M-blocks pack the same slot of both
            # heads ([q_h; q_h1] etc) so psum->sbuf copies are full 128-partition.
            for tci in range(2):
                tsl = slice(512 * tci, 512 * (tci + 1))
                for bi, (lo, dest) in enumerate([
                    (0, qq2[:, tsl]),
                    (64, kk2[:, S + 512 * tci: S + 512 * (tci + 1)]),
                    (128, vv2[:, S + 512 * tci: S + 512 * (tci + 1)]),
                ]):
                    mm_ps = psum.tile([128, 512], F32, tag="mm", bufs=2)
                    for kc in range(4):
                        nc.tensor.matmul(
                            mm_ps,
                            wq[:, kc, 384 * p + 128 * bi: 384 * p + 128 * (bi + 1)],
                            nrm[:, kc, tsl],
                            start=(kc == 0), stop=(kc == 3),
                        )
                    nc.vector.tensor_scalar_add(
                        out=dest, in0=mm_ps, scalar1=qb[:, p, bi:bi + 1])

            # encoder kv for the pair
            for bi, (lo, dest) in enumerate([(0, kk2[:, 0:S]), (64, vv2[:, 0:S])]):
                ek_ps = psum.tile([128, S], F32, tag="mm", bufs=2)
                for kc in range(4):
                    nc.tensor.matmul(
                        ek_ps,
                        we[:, kc, 256 * p + 128 * bi: 256 * p + 128 * (bi + 1)],
                        enc_sb[:, kc, :],
                        start=(kc == 0), stop=(kc == 3),
                    )
                nc.vector.tensor_scalar_add(
                    out=dest, in0=ek_ps, scalar1=eb[:, p, bi:bi + 1])

            # ones columns for the PV rowsum trick
            nc.gpsimd.memset(vT2[:, :, CH:CH + 1], 1.0)
            nc.gpsimd.memset(vT2[:, :, VTC + CH:VTC + CH + 1], 1.0)

            for hh in range(2):
                rlo = 64 * hh
                vcol = VTC * hh
                wgtT = hp.tile([128, NS, T], BF16, tag="wgtT")
                pv_ps = psum.tile([65, 2, 512], F32, tag="pv", bufs=1)

                for j in range(NS):
                    s0 = 128 * j
                    rows = min(128, ST - s0)
                    ssl = slice(s0, s0 + rows)
                    # v^T for this s-tile (PE transpose-mode)
                    tr_ps = psum.tile([128, 64], BF16, tag="mm", bufs=2)
                    nc.tensor.transpose(
                        tr_ps[0:rows, :], vv2[rlo:rlo + 64, ssl],
                        ident[rlo:rlo + 64, rlo:rlo + 64])
                    nc.vector.tensor_copy(
                        out=vT2[0:rows, j, vcol:vcol + 64], in_=tr_ps[0:rows, :])
                    # logits (transposed): wgtT[s, t] = k^T q
                    qk_ps = psum.tile([128, T], F32, tag="qk", bufs=2)
                    for tci in range(2):
                        nc.tensor.matmul(
                            qk_ps[0:rows, 512 * tci:512 * (tci + 1)],
                            kk2[rlo:rlo + 64, ssl],
                            qq2[rlo:rlo + 64, 512 * tci:512 * (tci + 1)],
                        )
                    # masked softmax numerator; mask is a per-partition bias
                    nc.scalar.activation(
                        out=wgtT[0:rows, j, :], in_=qk_ps[0:rows, :],
                        func=AF.Exp, scale=0.125,
                        bias=(addm[0:rows] if j == 0 else 0.0),
                    )

                # PV: out rows 0:64 = unnormalized attn out, row 64 = softmax denom
                for tci in range(2):
                    for j in range(NS):
                        rows = min(128, ST - 128 * j)
                        nc.tensor.matmul(
                            pv_ps[:, tci, :],
                            vT2[0:rows, j, vcol:vcol + 65],
                            wgtT[0:rows, j, 512 * tci:512 * (tci + 1)],
                            start=(j == 0), stop=(j == NS - 1),
                            skip_group_check=True,
                        )

                recipb = hp.tile([64, T], F32, tag="recipb")
                for tci in range(2):
                    nc.vector.reciprocal(
                        out=recipb[0:1, 512 * tci:512 * (tci + 1)],
                        in_=pv_ps[64:65, tci, :])
                nc.gpsimd.partition_broadcast(recipb, recipb[0:1, :])
                for tci in range(2):
                    nc.vector.tensor_mul(
                        out=att_all[rlo:rlo + 64, p, 512 * tci:512 * (tci + 1)],
                        in0=pv_ps[0:64, tci, :],
                        in1=recipb[:, 512 * tci:512 * (tci + 1)],
                    )

        # ---- output projection + residual ----------------------------------
        opool = ctx.enter_context(tc.tile_pool(name="opool", bufs=2))
        for i in range(4):
            for tci in range(2):
                tsl = slice(512 * tci, 512 * (tci + 1))
                pr_ps = psum.tile([128, 512], F32, tag="mm", bufs=2)
                for kc in range(4):
                    nc.tensor.matmul(
                        pr_ps, wp[:, kc, 128 * i:128 * (i + 1)],
                        att_all[:, kc, tsl],
                        start=(kc == 0), stop=(kc == 3),
                    )
                out_sb = opool.tile([128, 512], F32, tag="osb")
                nc.vector.scalar_tensor_tensor(
                    out=out_sb, in0=pr_ps, scalar=pb[:, i:i + 1],
                    in1=x_sb[:, i, tsl], op0=ALU.add, op1=ALU.add,
                )
                nc.sync.dma_start(out=out_d.ap()[:, i, tsl], in_=out_sb)

    nc.compile()
    return nc


def _to_part_major(a, inner):
    """[C, inner...] with C=512 -> [128, 4, inner] (c = 128*i + p)."""
    return np.ascontiguousarray(
        a.reshape(4, 128, inner).transpose(1, 0, 2))


def prep_inputs(x, encoder_out, capt_attn_mask, norm_scale, norm_bias,
                qkv_w, qkv_b, ekv_w, ekv_b, proj_w, proj_b):
    """Host-side marshalling: shard over batch + transpose/cast weights."""
    bf16 = ml_dtypes.bfloat16
    x = np.asarray(x, np.float32).reshape(B, C, T)
    enc = np.asarray(encoder_out, np.float32)
    mask = np.asarray(capt_attn_mask).astype(bool)

    x_dev = x.reshape(B, 4, 128, T).transpose(0, 2, 1, 3)
    enc_dev = enc.reshape(B, 4, 128, S).transpose(0, 2, 1, 3).astype(bf16)
    addm = np.zeros((B, 128, 1), np.float32)
    addm[:, :S, 0] = np.where(mask, 0.0, -10000.0)

    # weight rows permuted into per-pair block layout:
    # [q_h | q_h1 | k_h | k_h1 | v_h | v_h1] so each matmul lhsT is one
    # contiguous 128-column slice (walrus: single free dim only).
    qperm = np.array([
        192 * (2 * p + hh) + 64 * b + o
        for p in range(4) for b in range(3) for hh in range(2) for o in range(64)
    ])
    eperm = np.array([
        128 * (2 * p + hh) + 64 * b + o
        for p in range(4) for b in range(2) for hh in range(2) for o in range(64)
    ])
    wq_t = _to_part_major(np.asarray(qkv_w, np.float32)[qperm].T, 3 * C).astype(bf16)
    we_t = _to_part_major(np.asarray(ekv_w, np.float32)[eperm].T, 2 * C).astype(bf16)
    wp_t = _to_part_major(np.asarray(proj_w, np.float32).T, C).astype(bf16)

    qkv_b = np.asarray(qkv_b, np.float32)
    ekv_b = np.asarray(ekv_b, np.float32)
    qb = np.zeros((128, 4, 3), np.float32)
    ebb = np.zeros((128, 4, 2), np.float32)
    for p in range(4):
        h = 2 * p
        for bi in range(3):
            qb[0:64, p, bi] = qkv_b[192 * h + 64 * bi: 192 * h + 64 * bi + 64]
            qb[64:128, p, bi] = qkv_b[192 * (h + 1) + 64 * bi: 192 * (h + 1) + 64 * bi + 64]
        for bi in range(2):
            ebb[0:64, p, bi] = ekv_b[128 * h + 64 * bi: 128 * h + 64 * bi + 64]
            ebb[64:128, p, bi] = ekv_b[128 * (h + 1) + 64 * bi: 128 * (h + 1) + 64 * bi + 64]
    pbm = np.ascontiguousarray(np.asarray(proj_b, np.float32).reshape(4, 128).T)
    gamm = np.ascontiguousarray(np.asarray(norm_scale, np.float32).reshape(4, 128).T)
    betm = np.ascontiguousarray(np.asarray(norm_bias, np.float32).reshape(4, 128).T)

    shared = {"wq": wq_t, "we": we_t, "wp": wp_t, "qb": qb, "eb": ebb,
              "pb": pbm, "gam": gamm, "bet": betm}
    in_maps = []
    for b in range(B):
        m = dict(shared)
        m["x"] = np.ascontiguousarray(x_dev[b])
        m["enc"] = np.ascontiguousarray(enc_dev[b])
        m["addm"] = np.ascontiguousarray(addm[b])
        in_maps.append(m)
    return in_maps


def gather_output(results):
    out = np.stack([r["out"] for r in results])  # [8, 128, 4, T]
    return np.ascontiguousarray(
        out.transpose(0, 2, 1, 3).reshape(B, C, HH, WW).astype(np.float32))


_NC = None


def _get_nc():
    global _NC
    if _NC is None:
        _NC = build_program()
    return _NC


def kernel(**inputs) -> np.ndarray:
    from concourse.bass_utils import run_bass_kernel_spmd

    nc = _get_nc()
    in_maps = prep_inputs(**inputs)
    res = run_bass_kernel_spmd(nc, in_maps, core_ids=list(range(N_CORES)))
    return gather_output(res.results)


if __name__ == "__main__":
    nc = build_program()
    print("program built ok")


# revision 23
# speedup vs baseline: 5607.4756x; 5607.4756x over previous
"""AttentionBlock (GroupNorm32 + self/cross attention + proj + residual) on 8 TRN2 cores.

Sharding: data-parallel over batch. B=8 samples, one per NeuronCore. Each core runs
the full per-sample block: GroupNorm -> qkv/ekv projections -> 8-head attention
(encoder kv concat + additive mask) -> out projection -> residual.

Layout notes (per core):
  - channel-major tensors are stored as [128, 4, *]  with c = 128*i + p
  - attention logits are computed TRANSPOSED, wgtT[s, t], so the (per-s) additive
    mask becomes a per-partition ACT bias and softmax's exp is a single
    PSUM->SBUF activation pass; row sums come from an appended ones-column in
    v^T during the PV matmul (out row 64).
  - matmuls run in bf16 (fp32 PSUM accumulate).
"""

import sys
from contextlib import ExitStack

import numpy as np

for _p in ("/opt/trn_rl_repo",):
    if _p not in sys.path:
        sys.path.insert(0, _p)

import ml_dtypes  # noqa: E402

import concourse.bass as bass  # noqa: E402
import concourse.tile as tile  # noqa: E402
from concourse import bacc, mybir  # noqa: E402
from concourse.masks import make_identity  # noqa: E402

F32 = mybir.dt.float32
BF16 = mybir.dt.bfloat16
AF = mybir.ActivationFunctionType
ALU = mybir.AluOpType

B, C, HH, WW = 8, 512, 32, 32
T = HH * WW          # 1024
HEADS, CH, S = 8, 64, 77
ST = S + T           # 1101
NS = (ST + 127) // 128  # 9 s-tiles
GROUPS, GCH = 32, 16
N_CORES = 8

# v^T per head is [s, 65]: cols 0..63 = v channels, col 64 = ones (rowsum trick).
# Two heads per pair live side by side in one tile with a 66-col stride.
VTC = 66


def build_program():
    nc = bacc.Bacc("TRN2", target_bir_lowering=False, debug=False)

    x_d = nc.dram_tensor("x", [128, 4, T], F32, kind="ExternalInput")
    enc_d = nc.dram_tensor("enc", [128, 4, S], BF16, kind="ExternalInput")
    addm_d = nc.dram_tensor("addm", [128, 1], F32, kind="ExternalInput")
    wq_d = nc.dram_tensor("wq", [128, 4, 3 * C], BF16, kind="ExternalInput")
    we_d = nc.dram_tensor("we", [128, 4, 2 * C], BF16, kind="ExternalInput")
    wp_d = nc.dram_tensor("wp", [128, 4, C], BF16, kind="ExternalInput")
    qb_d = nc.dram_tensor("qb", [128, 4, 3], F32, kind="ExternalInput")
    eb_d = nc.dram_tensor("eb", [128, 4, 2], F32, kind="ExternalInput")
    pb_d = nc.dram_tensor("pb", [128, 4], F32, kind="ExternalInput")
    gam_d = nc.dram_tensor("gam", [128, 4], F32, kind="ExternalInput")
    bet_d = nc.dram_tensor("bet", [128, 4], F32, kind="ExternalInput")
    out_d = nc.dram_tensor("out", [128, 4, T], F32, kind="ExternalOutput")

    with tile.TileContext(nc) as tc, ExitStack() as ctx:
        consts = ctx.enter_context(tc.tile_pool(name="consts", bufs=1))
        stats = ctx.enter_context(tc.tile_pool(name="stats", bufs=1))
        hp = ctx.enter_context(tc.tile_pool(name="hp", bufs=2))
        psum = ctx.enter_context(tc.tile_pool(name="psum", bufs=1, space="PSUM"))

        # ---- constant loads -------------------------------------------------
        x_sb = consts.tile([128, 4, T], F32)
        nc.sync.dma_start(out=x_sb, in_=x_d.ap())
        enc_sb = consts.tile([128, 4, S], BF16)
        nc.sync.dma_start(out=enc_sb, in_=enc_d.ap())
        wq = consts.tile([128, 4, 3 * C], BF16)
        nc.sync.dma_start(out=wq, in_=wq_d.ap())
        we = consts.tile([128, 4, 2 * C], BF16)
        nc.sync.dma_start(out=we, in_=we_d.ap())
        wp = consts.tile([128, 4, C], BF16)
        nc.sync.dma_start(out=wp, in_=wp_d.ap())
        addm = consts.tile([128, 1], F32)
        nc.sync.dma_start(out=addm, in_=addm_d.ap())
        qb = consts.tile([128, 4, 3], F32)
        nc.sync.dma_start(out=qb, in_=qb_d.ap())
        eb = consts.tile([128, 4, 2], F32)
        nc.sync.dma_start(out=eb, in_=eb_d.ap())
        pb = consts.tile([128, 4], F32)
        nc.sync.dma_start(out=pb, in_=pb_d.ap())
        gam = consts.tile([128, 4], F32)
        nc.sync.dma_start(out=gam, in_=gam_d.ap())
        bet = consts.tile([128, 4], F32)
        nc.sync.dma_start(out=bet, in_=bet_d.ap())

        ident = consts.tile([128, 128], BF16)
        make_identity(nc, ident)
        identf = consts.tile([128, 128], F32)
        make_identity(nc, identf)

        # ---- GroupNorm(32) stats -------------------------------------------
        # per-channel mean/var via bn_stats, then aggregate channel groups of 16
        # across partitions via a PE transpose.
        mv = stats.tile([128, 4, 2], F32)
        for i in range(4):
            bnst = stats.tile([128, 2, 6], F32, tag="bnst", bufs=2)
            nc.vector.bn_stats(out=bnst[:, 0, :], in_=x_sb[:, i, 0:512])
            nc.vector.bn_stats(out=bnst[:, 1, :], in_=x_sb[:, i, 512:1024])
            nc.vector.bn_aggr(out=mv[:, i, :], in_=bnst)

        stm = stats.tile([128, 4], F32)
        nc.vector.tensor_copy(out=stm, in_=mv[:, :, 0])
        # E[x^2] = var + mean^2
        stx = stats.tile([128, 4], F32)
        nc.vector.tensor_mul(out=stx, in0=mv[:, :, 0], in1=mv[:, :, 0])
        nc.vector.tensor_add(out=stx, in0=stx, in1=mv[:, :, 1])

        stmT = stats.tile([4, 128], F32)
        stxT = stats.tile([4, 128], F32)
        for src_t, dst_t in ((stm, stmT), (stx, stxT)):
            t_ps = psum.tile([4, 128], F32, tag="mm", bufs=1, name=f"tps_{src_t.tensor.name}")
            nc.tensor.transpose(t_ps, src_t, identf)
            nc.vector.tensor_copy(out=dst_t, in_=t_ps)

        # group sums over 16 channels: [4, 8groups, 16ch] -> [4, 8]
        gm = stats.tile([4, 8], F32)
        gx = stats.tile([4, 8], F32)
        nc.vector.reduce_sum(
            out=gm, in_=stmT.rearrange("p (g k) -> p g k", k=GCH),
            axis=mybir.AxisListType.X)
        nc.vector.reduce_sum(
            out=gx, in_=stxT.rearrange("p (g k) -> p g k", k=GCH),
            axis=mybir.AxisListType.X)
        mug = stats.tile([4, 8], F32)
        nc.vector.tensor_scalar_mul(out=mug, in0=gm, scalar1=1.0 / GCH)
        varg = stats.tile([4, 8], F32)
        nc.vector.tensor_mul(out=varg, in0=mug, in1=mug)
        nc.vector.tensor_scalar_mul(out=gx, in0=gx, scalar1=1.0 / GCH)
        nc.vector.tensor_sub(out=varg, in0=gx, in1=varg)
        # rstd = (var+eps)^-1/2 = exp(-0.5*ln(var+eps)); keeps ACT on one table set
        eps_t = stats.tile([4, 1], F32)
        nc.vector.memset(eps_t, 1e-5)
        lnv = stats.tile([4, 8], F32)
        nc.scalar.activation(out=lnv, in_=varg, func=AF.Ln, bias=eps_t, scale=1.0)
        rstdg = stats.tile([4, 8], F32)
        nc.scalar.activation(out=rstdg, in_=lnv, func=AF.Exp, scale=-0.5)

        # transpose [4ctile, 8grp] -> [8grp, 4ctile], then replicate x16 along
        # partitions back to per-channel [128, 4] columns via a tiny DMA.
        mu_col = stats.tile([128, 4], F32)
        rstd_col = stats.tile([128, 4], F32)
        for src_t, dst_t in ((mug, mu_col), (rstdg, rstd_col)):
            t_ps = psum.tile([8, 4], F32, tag="mm", bufs=1, name=f"tbps_{src_t.tensor.name}")
            nc.tensor.transpose(t_ps, src_t, identf[0:4, 0:4])
            t_sb = stats.tile([8, 4], F32, name=f"tsb_{src_t.tensor.name}")
            nc.vector.tensor_copy(out=t_sb, in_=t_ps)
            nc.sync.dma_start(
                out=dst_t,
                in_=bass.AP(
                    tensor=t_sb.tensor, offset=t_sb.offset,
                    ap=[list(t_sb.ap[0]), [0, GCH], list(t_sb.ap[-1])],
                ),
            )

        a_col = stats.tile([128, 4], F32)
        nc.vector.tensor_mul(out=a_col, in0=rstd_col, in1=gam)
        b_col = stats.tile([128, 4], F32)
        nc.vector.tensor_mul(out=b_col, in0=mu_col, in1=a_col)
        nc.vector.tensor_sub(out=b_col, in0=bet, in1=b_col)

        nrm = consts.tile([128, 4, T], BF16)
        for i in range(4):
            nc.vector.tensor_scalar(
                out=nrm[:, i, :], in0=x_sb[:, i, :],
                scalar1=a_col[:, i:i + 1], scalar2=b_col[:, i:i + 1],
                op0=ALU.mult, op1=ALU.add,
            )

        att_all = consts.tile([128, 4, T], BF16)

        # ---- attention, two heads (one pair) at a time ----------------------
        for p in range(4):
            h0 = 2 * p
            qq2 = hp.tile([128, T], BF16, tag="qq2")
            kk2 = hp.tile([128, ST], BF16, tag="kk2")
            vv2 = hp.tile([128, ST], BF16, tag="vv2")
            vT2 = hp.tile([128, NS, 2 * VTC], BF16, tag="vT2")

            # qkv projection for the pair; M-blocks pack the same slot of both
            # heads ([q_h; q_h1] etc) so psum->sbuf copies are full 128-partition.
            for tci in range(2):
                tsl = slice(512 * tci, 512 * (tci + 1))
                for bi, (lo, dest) in enumerate([
                    (0, qq2[:, tsl]),
                    (64, kk2[:, S + 512 * tci: S + 512 * (tci + 1)]),
                    (128, vv2[:, S + 512 * tci: S + 512 * (tci + 1)]),
                ]):
                    mm_ps = psum.tile([128, 512], F32, tag="mm", bufs=1)
                    for kc in range(4):
                        nc.tensor.matmul(
                            mm_ps,
                            wq[:, kc, 384 * p + 128 * bi: 384 * p + 128 * (bi + 1)],
                            nrm[:, kc, tsl],
                            start=(kc == 0), stop=(kc == 3),
                        )
                    nc.vector.tensor_scalar_add(
                        out=dest, in0=mm_ps, scalar1=qb[:, p, bi:bi + 1])

            # encoder kv for the pair
            for bi, (lo, dest) in enumerate([(0, kk2[:, 0:S]), (64, vv2[:, 0:S])]):
                ek_ps = psum.tile([128, S], F32, tag="mm", bufs=1)
                for kc in range(4):
                    nc.tensor.matmul(
                        ek_ps,
                        we[:, kc, 256 * p + 128 * bi: 256 * p + 128 * (bi + 1)],
                        enc_sb[:, kc, :],
                        start=(kc == 0), stop=(kc == 3),
                    )
                nc.vector.tensor_scalar_add(
                    out=dest, in0=ek_ps, scalar1=eb[:, p, bi:bi + 1])

            # ones columns for the PV rowsum trick
            nc.gpsimd.memset(vT2[:, :, CH:CH + 1], 1.0)
            nc.gpsimd.memset(vT2[:, :, VTC + CH:VTC + CH + 1], 1.0)

            for hh in range(2):
                rlo = 64 * hh
                vcol = VTC * hh
                wgtT = hp.tile([128, NS, T], BF16, tag="wgtT")
                pv = [psum.tile([65, 512], F32, tag="pv", bufs=3,
                                name=f"pv_{p}_{hh}_{tci}") for tci in range(2)]

                for j in range(NS):
                    s0 = 128 * j
                    rows = min(128, ST - s0)
                    ssl = slice(s0, s0 + rows)
                    # v^T for this s-tile (PE transpose-mode)
                    tr_ps = psum.tile([128, 64], BF16, tag="mm", bufs=1)
                    nc.tensor.transpose(
                        tr_ps[0:rows, :], vv2[rlo:rlo + 64, ssl],
                        ident[rlo:rlo + 64, rlo:rlo + 64])
                    nc.vector.tensor_copy(
                        out=vT2[0:rows, j, vcol:vcol + 64], in_=tr_ps[0:rows, :])
                    # logits (transposed): wgtT[s, t] = k^T q
                    qk_ps = psum.tile([128, T], F32, tag="qk", bufs=2)
                    for tci in range(2):
                        nc.tensor.matmul(
                            qk_ps[0:rows, 512 * tci:512 * (tci + 1)],
                            kk2[rlo:rlo + 64, ssl],
                            qq2[rlo:rlo + 64, 512 * tci:512 * (tci + 1)],
                        )
                    # masked softmax numerator; mask is a per-partition bias
                    nc.scalar.activation(
                        out=wgtT[0:rows, j, :], in_=qk_ps[0:rows, :],
                        func=AF.Exp, scale=0.125,
                        bias=(addm[0:rows] if j == 0 else 0.0),
                    )

                # PV: out rows 0:64 = unnormalized attn out, row 64 = softmax denom
                for tci in range(2):
                    for j in range(NS):
                        rows = min(128, ST - 128 * j)
                        nc.tensor.matmul(
                            pv[tci],
                            vT2[0:rows, j, vcol:vcol + 65],
                            wgtT[0:rows, j, 512 * tci:512 * (tci + 1)],
                            start=(j == 0), stop=(j == NS - 1),
                            skip_group_check=True,
                        )

                # softmax denominators -> SBUF, then approx reciprocal there
                # (the custom-DVE op needs SBUF operands), broadcast, scale.
                sums = hp.tile([1, T], F32, tag="sums")
                recipb = hp.tile([64, T], F32, tag="recipb")
                for tci in range(2):
                    nc.vector.tensor_copy(
                        out=sums[0:1, 512 * tci:512 * (tci + 1)],
                        in_=pv[tci][64:65, :])
                nc.vector.reciprocal_approx_fast(out=recipb[0:1, :], in_=sums)
                nc.gpsimd.partition_broadcast(recipb, recipb[0:1, :])
                for tci in range(2):
                    nc.vector.tensor_mul(
                        out=att_all[rlo:rlo + 64, p, 512 * tci:512 * (tci + 1)],
                        in0=pv[tci][0:64, :],
                        in1=recipb[:, 512 * tci:512 * (tci + 1)],
                    )

        # ---- output projection + residual ----------------------------------
        opool = ctx.enter_context(tc.tile_pool(name="opool", bufs=2))
        for i in range(4):
            for tci in range(2):
                tsl = slice(512 * tci, 512 * (tci + 1))
                pr_ps = psum.tile([128, 512], F32, tag="mm", bufs=1)
                for kc in range(4):
                    nc.tensor.matmul(
                        pr_ps, wp[:, kc, 128 * i:128 * (i + 1)],
                        att_all[:, kc, tsl],
                        start=(kc == 0), stop=(kc == 3),
                    )
                out_sb = opool.tile([128, 512], F32, tag="osb")
                nc.vector.scalar_tensor_tensor(
                    out=out_sb, in0=pr_ps, scalar=pb[:, i:i + 1],
                    in1=x_sb[:, i, tsl], op0=ALU.add, op1=ALU.add,
                )
                nc.sync.dma_start(out=out_d.ap()[:, i, tsl], in_=out_sb)

    nc.compile()
    return nc


def _to_part_major(a, inner):
    """[C, inner...] with C=512 -> [128, 4, inner] (c = 128*i + p)."""
    return np.ascontiguousarray(
        a.reshape(4, 128, inner).transpose(1, 0, 2))


def prep_inputs(x, encoder_out, capt_attn_mask, norm_scale, norm_bias,
                qkv_w, qkv_b, ekv_w, ekv_b, proj_w, proj_b):
    """Host-side marshalling: shard over batch + transpose/cast weights."""
    bf16 = ml_dtypes.bfloat16
    x = np.asarray(x, np.float32).reshape(B, C, T)
    enc = np.asarray(encoder_out, np.float32)
    mask = np.asarray(capt_attn_mask).astype(bool)

    x_dev = x.reshape(B, 4, 128, T).transpose(0, 2, 1, 3)
    enc_dev = enc.reshape(B, 4, 128, S).transpose(0, 2, 1, 3).astype(bf16)
    addm = np.zeros((B, 128, 1), np.float32)
    addm[:, :S, 0] = np.where(mask, 0.0, -10000.0)

    # weight rows permuted into per-pair block layout:
    # [q_h | q_h1 | k_h | k_h1 | v_h | v_h1] so each matmul lhsT is one
    # contiguous 128-column slice (walrus: single free dim only).
    qperm = np.array([
        192 * (2 * p + hh) + 64 * b + o
        for p in range(4) for b in range(3) for hh in range(2) for o in range(64)
    ])
    eperm = np.array([
        128 * (2 * p + hh) + 64 * b + o
        for p in range(4) for b in range(2) for hh in range(2) for o in range(64)
    ])
    wq_t = _to_part_major(np.asarray(qkv_w, np.float32)[qperm].T, 3 * C).astype(bf16)
    we_t = _to_part_major(np.asarray(ekv_w, np.float32)[eperm].T, 2 * C).astype(bf16)
    wp_t = _to_part_major(np.asarray(proj_w, np.float32).T, C).astype(bf16)

    qkv_b = np.asarray(qkv_b, np.float32)
    ekv_b = np.asarray(ekv_b, np.float32)
    qb = np.zeros((128, 4, 3), np.float32)
    ebb = np.zeros((128, 4, 2), np.float32)
    for p in range(4):
        h = 2 * p
        for bi in range(3):
            qb[0:64, p, bi] = qkv_b[192 * h + 64 * bi: 192 * h + 64 * bi + 64]
            qb[64:128, p, bi] = qkv_b[192 * (h + 1) + 64 * bi: 192 * (h + 1) + 64 * bi + 64]
        for bi in range(2):
            ebb[0:64, p, bi] = ekv_b[128 * h + 64 * bi: 128 * h + 64 * bi + 64]
            ebb[64:128, p, bi] = ekv_b[128 * (h + 1) + 64 * bi: 128 * (h + 1) + 64 * bi + 64]
    pbm = np.ascontiguousarray(np.asarray(proj_b, np.float32).reshape(4, 128).T)
    gamm = np.ascontiguousarray(np.asarray(norm_scale, np.float32).reshape(4, 128).T)
    betm = np.ascontiguousarray(np.asarray(norm_bias, np.float32).reshape(4, 128).T)

    shared = {"wq": wq_t, "we": we_t, "wp": wp_t, "qb": qb, "eb": ebb,
              "pb": pbm, "gam": gamm, "bet": betm}
    in_maps = []
    for b in range(B):
        m = dict(shared)
        m["x"] = np.ascontiguousarray(x_dev[b])
        m["enc"] = np.ascontiguousarray(enc_dev[b])
        m["addm"] = np.ascontiguousarray(addm[b])
        in_maps.append(m)
    return in_maps


def gather_output(results):
    out = np.stack([r["out"] for r in results])  # [8, 128, 4, T]
    return np.ascontiguousarray(
        out.transpose(0, 2, 1, 3).reshape(B, C, HH, WW).astype(np.float32))


_NC = None


def _get_nc():
    global _NC
    if _NC is None:
        _NC = build_program()
    return _NC


def kernel(**inputs) -> np.ndarray:
    from concourse.bass_utils import run_bass_kernel_spmd

    nc = _get_nc()
    in_maps = prep_inputs(**inputs)
    res = run_bass_kernel_spmd(nc, in_maps, core_ids=list(range(N_CORES)))
    return gather_output(res.results)


if __name__ == "__main__":
    nc = build_program()
    print("program built ok")


# revision 25
# speedup vs baseline: 7655.5686x; 1.3652x over previous
"""AttentionBlock (GroupNorm32 + self/cross attention + proj + residual) on 8 TRN2 cores.

Sharding: data-parallel over batch. B=8 samples, one per NeuronCore. Each core runs
the full per-sample block: GroupNorm -> qkv/ekv projections -> 8-head attention
(encoder kv concat + additive mask) -> out projection -> residual.

Layout notes (per core):
  - channel-major tensors are stored as [128, 4, *]  with c = 128*i + p
  - attention logits are computed TRANSPOSED, wgtT[s, t], so the (per-s) additive
    mask becomes a per-partition ACT bias and softmax's exp is a single
    PSUM->SBUF activation pass; row sums come from an appended ones-column in
    v^T during the PV matmul (out row 64).
  - matmuls run in bf16 (fp32 PSUM accumulate).
"""

import sys
from contextlib import ExitStack

import numpy as np

for _p in ("/opt/trn_rl_repo",):
    if _p not in sys.path:
        sys.path.insert(0, _p)

import ml_dtypes  # noqa: E402

import concourse.bass as bass  # noqa: E402
import concourse.tile as tile  # noqa: E402
from concourse import bacc, mybir  # noqa: E402
from concourse.masks import make_identity  # noqa: E402

F32 = mybir.dt.float32
BF16 = mybir.dt.bfloat16
AF = mybir.ActivationFunctionType
ALU = mybir.AluOpType

B, C, HH, WW = 8, 512, 32, 32
T = HH * WW          # 1024
HEADS, CH, S = 8, 64, 77
ST = S + T           # 1101
NS = (ST + 127) // 128  # 9 s-tiles
GROUPS, GCH = 32, 16
N_CORES = 8

# v^T per head is [s, 65]: cols 0..63 = v channels, col 64 = ones (rowsum trick).
# Two heads per pair live side by side in one tile with a 66-col stride.
VTC = 66


def build_program():
    nc = bacc.Bacc("TRN2", target_bir_lowering=False, debug=False)

    x_d = nc.dram_tensor("x", [128, 4, T], F32, kind="ExternalInput")
    enc_d = nc.dram_tensor("enc", [128, 4, S], BF16, kind="ExternalInput")
    addm_d = nc.dram_tensor("addm", [128, 1], F32, kind="ExternalInput")
    wq_d = nc.dram_tensor("wq", [128, 4, 3 * C], BF16, kind="ExternalInput")
    we_d = nc.dram_tensor("we", [128, 4, 2 * C], BF16, kind="ExternalInput")
    wp_d = nc.dram_tensor("wp", [128, 4, C], BF16, kind="ExternalInput")
    qb_d = nc.dram_tensor("qb", [128, 4, 3], F32, kind="ExternalInput")
    eb_d = nc.dram_tensor("eb", [128, 4, 2], F32, kind="ExternalInput")
    pb_d = nc.dram_tensor("pb", [128, 4], F32, kind="ExternalInput")
    gam_d = nc.dram_tensor("gam", [128, 4], F32, kind="ExternalInput")
    bet_d = nc.dram_tensor("bet", [128, 4], F32, kind="ExternalInput")
    out_d = nc.dram_tensor("out", [128, 4, T], F32, kind="ExternalOutput")

    with tile.TileContext(nc) as tc, ExitStack() as ctx:
        consts = ctx.enter_context(tc.tile_pool(name="consts", bufs=1))
        stats = ctx.enter_context(tc.tile_pool(name="stats", bufs=1))
        hp = ctx.enter_context(tc.tile_pool(name="hp", bufs=2))
        psum = ctx.enter_context(tc.tile_pool(name="psum", bufs=1, space="PSUM"))

        # ---- constant loads -------------------------------------------------
        x_sb = consts.tile([128, 4, T], F32)
        nc.sync.dma_start(out=x_sb, in_=x_d.ap())
        enc_sb = consts.tile([128, 4, S], BF16)
        nc.sync.dma_start(out=enc_sb, in_=enc_d.ap())
        wq = consts.tile([128, 4, 3 * C], BF16)
        nc.sync.dma_start(out=wq, in_=wq_d.ap())
        we = consts.tile([128, 4, 2 * C], BF16)
        nc.sync.dma_start(out=we, in_=we_d.ap())
        wp = consts.tile([128, 4, C], BF16)
        nc.sync.dma_start(out=wp, in_=wp_d.ap())
        addm = consts.tile([128, 1], F32)
        nc.sync.dma_start(out=addm, in_=addm_d.ap())
        qb = consts.tile([128, 4, 3], F32)
        nc.sync.dma_start(out=qb, in_=qb_d.ap())
        eb = consts.tile([128, 4, 2], F32)
        nc.sync.dma_start(out=eb, in_=eb_d.ap())
        pb = consts.tile([128, 4], F32)
        nc.sync.dma_start(out=pb, in_=pb_d.ap())
        gam = consts.tile([128, 4], F32)
        nc.sync.dma_start(out=gam, in_=gam_d.ap())
        bet = consts.tile([128, 4], F32)
        nc.sync.dma_start(out=bet, in_=bet_d.ap())

        identf = consts.tile([128, 128], F32)
        make_identity(nc, identf)
        ident = consts.tile([128, 128], BF16)
        make_identity(nc, ident)

        # ---- GroupNorm(32) stats -------------------------------------------
        # per-channel mean/var via bn_stats, then aggregate channel groups of 16
        # across partitions via a PE transpose.
        mv = stats.tile([128, 4, 2], F32)
        for i in range(4):
            bnst = stats.tile([128, 2, 6], F32, tag="bnst", bufs=2)
            nc.vector.bn_stats(out=bnst[:, 0, :], in_=x_sb[:, i, 0:512])
            nc.vector.bn_stats(out=bnst[:, 1, :], in_=x_sb[:, i, 512:1024])
            nc.vector.bn_aggr(out=mv[:, i, :], in_=bnst)

        stm = stats.tile([128, 4], F32)
        nc.vector.tensor_copy(out=stm, in_=mv[:, :, 0])
        # E[x^2] = var + mean^2
        stx = stats.tile([128, 4], F32)
        nc.vector.tensor_mul(out=stx, in0=mv[:, :, 0], in1=mv[:, :, 0])
        nc.vector.tensor_add(out=stx, in0=stx, in1=mv[:, :, 1])

        stmT = stats.tile([4, 128], F32)
        stxT = stats.tile([4, 128], F32)
        for src_t, dst_t in ((stm, stmT), (stx, stxT)):
            t_ps = psum.tile([4, 128], F32, tag="mm", bufs=2, name=f"tps_{src_t.tensor.name}")
            nc.tensor.transpose(t_ps, src_t, identf)
            nc.vector.tensor_copy(out=dst_t, in_=t_ps)

        # group sums over 16 channels: [4, 8groups, 16ch] -> [4, 8]
        gm = stats.tile([4, 8], F32)
        gx = stats.tile([4, 8], F32)
        nc.vector.reduce_sum(
            out=gm, in_=stmT.rearrange("p (g k) -> p g k", k=GCH),
            axis=mybir.AxisListType.X)
        nc.vector.reduce_sum(
            out=gx, in_=stxT.rearrange("p (g k) -> p g k", k=GCH),
            axis=mybir.AxisListType.X)
        mug = stats.tile([4, 8], F32)
        nc.vector.tensor_scalar_mul(out=mug, in0=gm, scalar1=1.0 / GCH)
        varg = stats.tile([4, 8], F32)
        nc.vector.tensor_mul(out=varg, in0=mug, in1=mug)
        nc.vector.tensor_scalar_mul(out=gx, in0=gx, scalar1=1.0 / GCH)
        nc.vector.tensor_sub(out=varg, in0=gx, in1=varg)
        # rstd = (var+eps)^-1/2 = exp(-0.5*ln(var+eps)); keeps ACT on one table set
        eps_t = stats.tile([4, 1], F32)
        nc.vector.memset(eps_t, 1e-5)
        lnv = stats.tile([4, 8], F32)
        nc.scalar.activation(out=lnv, in_=varg, func=AF.Ln, bias=eps_t, scale=1.0)
        rstdg = stats.tile([4, 8], F32)
        nc.scalar.activation(out=rstdg, in_=lnv, func=AF.Exp, scale=-0.5)

        # transpose [4ctile, 8grp] -> [8grp, 4ctile], then replicate x16 along
        # partitions back to per-channel [128, 4] columns via a tiny DMA.
        mu_col = stats.tile([128, 4], F32)
        rstd_col = stats.tile([128, 4], F32)
        for src_t, dst_t in ((mug, mu_col), (rstdg, rstd_col)):
            t_ps = psum.tile([8, 4], F32, tag="mm", bufs=2, name=f"tbps_{src_t.tensor.name}")
            nc.tensor.transpose(t_ps, src_t, identf[0:4, 0:4])
            t_sb = stats.tile([8, 4], F32, name=f"tsb_{src_t.tensor.name}")
            nc.vector.tensor_copy(out=t_sb, in_=t_ps)
            nc.sync.dma_start(
                out=dst_t,
                in_=bass.AP(
                    tensor=t_sb.tensor, offset=t_sb.offset,
                    ap=[list(t_sb.ap[0]), [0, GCH], list(t_sb.ap[-1])],
                ),
            )

        a_col = stats.tile([128, 4], F32)
        nc.vector.tensor_mul(out=a_col, in0=rstd_col, in1=gam)
        b_col = stats.tile([128, 4], F32)
        nc.vector.tensor_mul(out=b_col, in0=mu_col, in1=a_col)
        nc.vector.tensor_sub(out=b_col, in0=bet, in1=b_col)

        nrm = consts.tile([128, 4, T], BF16)
        for i in range(4):
            nc.vector.tensor_scalar(
                out=nrm[:, i, :], in0=x_sb[:, i, :],
                scalar1=a_col[:, i:i + 1], scalar2=b_col[:, i:i + 1],
                op0=ALU.mult, op1=ALU.add,
            )

        att_all = consts.tile([128, 4, T], BF16)

        # ---- attention, two heads (one pair) at a time ----------------------
        for p in range(4):
            h0 = 2 * p
            qq2 = hp.tile([128, T], BF16, tag="qq2")
            kk2 = hp.tile([128, ST], BF16, tag="kk2")
            vv2 = hp.tile([128, ST], BF16, tag="vv2")
            vT2 = hp.tile([128, NS, 2 * VTC], BF16, tag="vT2")

            # qkv projection for the pair; M-blocks pack the same slot of both
            # heads ([q_h; q_h1] etc) so psum->sbuf copies are full 128-partition.
            for tci in range(2):
                tsl = slice(512 * tci, 512 * (tci + 1))
                for bi, (lo, dest) in enumerate([
                    (0, qq2[:, tsl]),
                    (64, kk2[:, S + 512 * tci: S + 512 * (tci + 1)]),
                    (128, vv2[:, S + 512 * tci: S + 512 * (tci + 1)]),
                ]):
                    mm_ps = psum.tile([128, 512], F32, tag="mm", bufs=2)
                    for kc in range(4):
                        nc.tensor.matmul(
                            mm_ps,
                            wq[:, kc, 384 * p + 128 * bi: 384 * p + 128 * (bi + 1)],
                            nrm[:, kc, tsl],
                            start=(kc == 0), stop=(kc == 3),
                        )
                    nc.vector.tensor_scalar_add(
                        out=dest, in0=mm_ps, scalar1=qb[:, p, bi:bi + 1])

            # encoder kv for the pair
            for bi, (lo, dest) in enumerate([(0, kk2[:, 0:S]), (64, vv2[:, 0:S])]):
                ek_ps = psum.tile([128, S], F32, tag="mm", bufs=2)
                for kc in range(4):
                    nc.tensor.matmul(
                        ek_ps,
                        we[:, kc, 256 * p + 128 * bi: 256 * p + 128 * (bi + 1)],
                        enc_sb[:, kc, :],
                        start=(kc == 0), stop=(kc == 3),
                    )
                nc.vector.tensor_scalar_add(
                    out=dest, in0=ek_ps, scalar1=eb[:, p, bi:bi + 1])

            # v^T via PE transpose-mode: one [128, chunk] transpose covers
            # BOTH heads' v rows (partitions 0:64 = v_h, 64:128 = v_h1).
            nc.gpsimd.memset(vT2[:, :, CH:CH + 1], 1.0)
            nc.gpsimd.memset(vT2[:, :, VTC + CH:VTC + CH + 1], 1.0)
            for j in range(NS):
                s0 = 128 * j
                rows = min(128, ST - s0)
                tr_ps = psum.tile([128, 128], BF16, tag="mm", bufs=2,
                                  name=f"tr_{p}_{j}")
                nc.tensor.transpose(
                    tr_ps[0:rows, :], vv2[:, s0:s0 + rows], ident)
                nc.vector.tensor_copy(
                    out=vT2[0:rows, j, 0:CH], in_=tr_ps[0:rows, 0:64])
                nc.vector.tensor_copy(
                    out=vT2[0:rows, j, VTC:VTC + CH], in_=tr_ps[0:rows, 64:128])

            # logits (transposed): wgtT[s, t] = k^T q. Emission interleaves
            # the two heads (disjoint PE row groups, K=64 at offsets 0/64) so
            # adjacent matmuls can overlap on the array.
            wgts = {}
            for hh in range(2):
                wgts[hh] = hp.tile([128, NS, T], BF16, tag="wgtT", bufs=3,
                                   name=f"wgtT_{p}_{hh}")
            for j in range(NS):
                s0 = 128 * j
                rows = min(128, ST - s0)
                ssl = slice(s0, s0 + rows)
                qk = [psum.tile([128, T], F32, tag="qk", bufs=2,
                                name=f"qk_{p}_{j}_{hh}") for hh in range(2)]
                for tci in range(2):
                    for hh in range(2):
                        rlo = 64 * hh
                        nc.tensor.matmul(
                            qk[hh][0:rows, 512 * tci:512 * (tci + 1)],
                            kk2[rlo:rlo + 64, ssl],
                            qq2[rlo:rlo + 64, 512 * tci:512 * (tci + 1)],
                        )
                for hh in range(2):
                    nc.scalar.activation(
                        out=wgts[hh][0:rows, j, :], in_=qk[hh][0:rows, :],
                        func=AF.Exp, scale=0.125,
                        bias=(addm[0:rows] if j == 0 else 0.0),
                    )

            for hh in range(2):
                rlo = 64 * hh
                vcol = VTC * hh
                wgtT = wgts[hh]
                # PV: out rows 0:64 = unnormalized attn out, row 64 = denom
                pv_ps = psum.tile([65, 2, 512], F32, tag="pv", bufs=1)
                for tci in range(2):
                    for j in range(NS):
                        rows = min(128, ST - 128 * j)
                        nc.tensor.matmul(
                            pv_ps[:, tci, :],
                            vT2[0:rows, j, vcol:vcol + 65],
                            wgtT[0:rows, j, 512 * tci:512 * (tci + 1)],
                            start=(j == 0), stop=(j == NS - 1),
                            skip_group_check=True,
                        )

                # softmax denominators -> SBUF first (the custom-DVE approx
                # reciprocal needs SBUF operands), then broadcast + scale.
                sums = hp.tile([1, T], F32, tag="sums")
                recipb = hp.tile([64, T], F32, tag="recipb")
                for tci in range(2):
                    nc.vector.tensor_copy(
                        out=sums[0:1, 512 * tci:512 * (tci + 1)],
                        in_=pv_ps[64:65, tci, :])
                nc.vector.reciprocal_approx_fast(out=recipb[0:1, :], in_=sums)
                nc.gpsimd.partition_broadcast(recipb, recipb[0:1, :])
                for tci in range(2):
                    nc.vector.tensor_mul(
                        out=att_all[rlo:rlo + 64, p, 512 * tci:512 * (tci + 1)],
                        in0=pv_ps[0:64, tci, :],
                        in1=recipb[:, 512 * tci:512 * (tci + 1)],
                    )

        # ---- output projection + residual ----------------------------------
        opool = ctx.enter_context(tc.tile_pool(name="opool", bufs=2))
        for i in range(4):
            for tci in range(2):
                tsl = slice(512 * tci, 512 * (tci + 1))
                pr_ps = psum.tile([128, 512], F32, tag="mm", bufs=2)
                for kc in range(4):
                    nc.tensor.matmul(
                        pr_ps, wp[:, kc, 128 * i:128 * (i + 1)],
                        att_all[:, kc, tsl],
                        start=(kc == 0), stop=(kc == 3),
                    )
                out_sb = opool.tile([128, 512], F32, tag="osb")
                nc.vector.scalar_tensor_tensor(
                    out=out_sb, in0=pr_ps, scalar=pb[:, i:i + 1],
                    in1=x_sb[:, i, tsl], op0=ALU.add, op1=ALU.add,
                )
                nc.sync.dma_start(out=out_d.ap()[:, i, tsl], in_=out_sb)

    nc.compile()
    return nc


def _to_part_major(a, inner):
    """[C, inner...] with C=512 -> [128, 4, inner] (c = 128*i + p)."""
    return np.ascontiguousarray(
        a.reshape(4, 128, inner).transpose(1, 0, 2))


def prep_inputs(x, encoder_out, capt_attn_mask, norm_scale, norm_bias,
                qkv_w, qkv_b, ekv_w, ekv_b, proj_w, proj_b):
    """Host-side marshalling: shard over batch + transpose/cast weights."""
    bf16 = ml_dtypes.bfloat16
    x = np.asarray(x, np.float32).reshape(B, C, T)
    enc = np.asarray(encoder_out, np.float32)
    mask = np.asarray(capt_attn_mask).astype(bool)

    x_dev = x.reshape(B, 4, 128, T).transpose(0, 2, 1, 3)
    enc_dev = enc.reshape(B, 4, 128, S).transpose(0, 2, 1, 3).astype(bf16)
    addm = np.zeros((B, 128, 1), np.float32)
    addm[:, :S, 0] = np.where(mask, 0.0, -10000.0)

    # weight rows permuted into per-pair block layout:
    # [q_h | q_h1 | k_h | k_h1 | v_h | v_h1] so each matmul lhsT is one
    # contiguous 128-column slice (walrus: single free dim only).
    qperm = np.array([
        192 * (2 * p + hh) + 64 * b + o
        for p in range(4) for b in range(3) for hh in range(2) for o in range(64)
    ])
    eperm = np.array([
        128 * (2 * p + hh) + 64 * b + o
        for p in range(4) for b in range(2) for hh in range(2) for o in range(64)
    ])
    wq_t = _to_part_major(np.asarray(qkv_w, np.float32)[qperm].T, 3 * C).astype(bf16)
    we_t = _to_part_major(np.asarray(ekv_w, np.float32)[eperm].T, 2 * C).astype(bf16)
    wp_t = _to_part_major(np.asarray(proj_w, np.float32).T, C).astype(bf16)

    qkv_b = np.asarray(qkv_b, np.float32)
    ekv_b = np.asarray(ekv_b, np.float32)
    qb = np.zeros((128, 4, 3), np.float32)
    ebb = np.zeros((128, 4, 2), np.float32)
    for p in range(4):
        h = 2 * p
        for bi in range(3):
            qb[0:64, p, bi] = qkv_b[192 * h + 64 * bi: 192 * h + 64 * bi + 64]
            qb[64:128, p, bi] = qkv_b[192 * (h + 1) + 64 * bi: 192 * (h + 1) + 64 * bi + 64]
        for bi in range(2):
            ebb[0:64, p, bi] = ekv_b[128 * h + 64 * bi: 128 * h + 64 * bi + 64]
            ebb[64:128, p, bi] = ekv_b[128 * (h + 1) + 64 * bi: 128 * (h + 1) + 64 * bi + 64]
    pbm = np.ascontiguousarray(np.asarray(proj_b, np.float32).reshape(4, 128).T)
    gamm = np.ascontiguousarray(np.asarray(norm_scale, np.float32).reshape(4, 128).T)
    betm = np.ascontiguousarray(np.asarray(norm_bias, np.float32).reshape(4, 128).T)

    shared = {"wq": wq_t, "we": we_t, "wp": wp_t, "qb": qb, "eb": ebb,
              "pb": pbm, "gam": gamm, "bet": betm}
    in_maps = []
    for b in range(B):
        m = dict(shared)
        m["x"] = np.ascontiguousarray(x_dev[b])
        m["enc"] = np.ascontiguousarray(enc_dev[b])
        m["addm"] = np.ascontiguousarray(addm[b])
        in_maps.append(m)
    return in_maps


def gather_output(results):
    out = np.stack([r["out"] for r in results])  # [8, 128, 4, T]
    return np.ascontiguousarray(
        out.transpose(0, 2, 1, 3).reshape(B, C, HH, WW).astype(np.float32))


_NC = None


def _get_nc():
    global _NC
    if _NC is None:
        _NC = build_program()
    return _NC


def kernel(**inputs) -> np.ndarray:
    from concourse.bass_utils import run_bass_kernel_spmd

    nc = _get_nc()
    in_maps = prep_inputs(**inputs)
    res = run_bass_kernel_spmd(nc, in_maps, core_ids=list(range(N_CORES)))
    return gather_output(res.results)


if __name__ == "__main__":
    nc = build_program()
    print("program built ok")
